# revision 23
# baseline (speedup 1.0000x reference)
"""Trainium2 Bass kernel for CohereAttention (T=2048, H=4096, NH=32, NKV=8, HD=128).

Sharding: tensor-parallel across heads on 8 cores (SGLang-style).
  - core c owns q-heads [4c, 4c+4) and kv-head c (GQA rep=4 maps exactly).
  - w_qkv column-sharded per core -> [4096, 768] (4q|1k|1v head blocks).
  - attention output (bf16, d-major [512, 2048]) AllGather'd across cores.
  - w_o column-sharded -> each core computes a [2048, 512] column shard of the
    output (stored transposed [512, 2048]); host concatenates.

Device pipeline per core (engine-decoupled, one attention head trickled per
token-tile section so the PE never starves on the DVE layernorm chain):
  section tt: qkv MMs (PSUM, single buffer) -> V cast (gpsimd) ->
      LN+RoPE chain (DVE only; mean/var scale and sqrt folded into
      scalar_tensor_tensor / tensor_scalar-pow, sign+sqrt(HD) folded into the
      host-prepped lnw) -> one attention head of the newest ready q-block
      (scores MM -> exp on Scalar -> causal mask on gpsimd -> PV/sums MMs,
      epilogue: reciprocal_approx_fast on DVE + normalize mult on gpsimd) ->
      PE transposes of q/k tiles -> (AllGather when a block's 4 heads done).
  tail: w_o load, last block's 4 heads, AllGather, o_proj per gathered block.
"""

import numpy as np
import ml_dtypes

T = 2048
H = 4096
NH = 32
NKV = 8
HD = 128
N_CORES = 8
QH = NH // N_CORES          # q heads per core = 4
LNH = QH + 1                # layernormed heads per core (4 q + 1 k)
EPS = 1e-5
THETA = 10000.0
SCALE = HD ** -0.5
TT = T // 128               # 16 token tiles
KO = H // 128               # 32 contraction chunks
QC = T // 512               # 4 query chunks of 512
BF16 = ml_dtypes.bfloat16

# section -> attention heads (qc, h) to emit there; block 2 front-loaded
HEAD_SCHED = {
    4: [(0, 0)], 5: [(0, 1)], 6: [(0, 2)], 7: [(0, 3)],
    8: [(1, 0)], 9: [(1, 1)], 10: [(1, 2)], 11: [(1, 3)],
    12: [(2, 0), (2, 1)], 13: [(2, 2), (2, 3)],
}

_CACHE = {}


def _build():
    import concourse.bass as bass
    import concourse.mybir as mybir
    import concourse.tile as tile
    from concourse import bacc
    from contextlib import ExitStack

    dt = mybir.dt
    f32 = dt.float32
    bf16 = dt.bfloat16
    AX = mybir.AxisListType
    OP = mybir.AluOpType
    ACT = mybir.ActivationFunctionType

    nc = bacc.Bacc("TRN2", target_bir_lowering=False, debug=False,
                   num_devices=N_CORES)

    # ---- I/O ----
    hT = nc.dram_tensor("hT", [TT, 128, KO, 128], bf16, kind="ExternalInput")
    wqkv = nc.dram_tensor("wqkv", [H, 768], bf16, kind="ExternalInput")
    wo = nc.dram_tensor("wo", [H, 512], bf16, kind="ExternalInput")
    cosd = nc.dram_tensor("cosd", [128, TT, 64], f32, kind="ExternalInput")
    sind = nc.dram_tensor("sind", [128, TT, 64], f32, kind="ExternalInput")
    lnw = nc.dram_tensor("lnw", [128, LNH, 128], f32, kind="ExternalInput")
    triu = nc.dram_tensor("triu", [128, 128], bf16, kind="ExternalInput")
    ident = nc.dram_tensor("ident", [128, 128], bf16, kind="ExternalInput")
    onesd = nc.dram_tensor("onesd", [128, 128], bf16, kind="ExternalInput")
    outT = nc.dram_tensor("outT", [512, T], f32, kind="ExternalOutput")

    with tile.TileContext(nc) as tc, ExitStack() as ctx:
        const = ctx.enter_context(tc.tile_pool(name="const", bufs=1))
        dram = ctx.enter_context(tc.tile_pool(name="dram", bufs=1, space="DRAM"))

        ag_in = [dram.tile([QH * 128, 512], bf16, name=f"agi{i}")
                 for i in range(QC)]
        ag_out = [dram.tile([NH * 128, 512], bf16, addr_space="Shared",
                            name=f"ago{i}")
                  for i in range(QC)]
        # the last block's gather is split in two 2-head halves so o_proj's
        # final block can start ~half an AllGather earlier
        ag3_in = [dram.tile([2 * 128, 512], bf16, name=f"agi3{i}")
                  for i in range(2)]
        ag3_out = [dram.tile([N_CORES * 2 * 128, 512], bf16,
                             addr_space="Shared", name=f"ago3{i}")
                   for i in range(2)]

        with tc.tile_pool(name="sps", bufs=2, space="PSUM") as sps, \
             tc.tile_pool(name="pvp", bufs=2, space="PSUM") as pvp, \
             tc.tile_pool(name="smp", bufs=2, space="PSUM") as smp, \
             tc.tile_pool(name="probs", bufs=3) as probs, \
             tc.tile_pool(name="attn", bufs=3) as attn, \
             tc.tile_pool(name="acts", bufs=1) as acts:

            # persistent activations: d-major Q/K, t-major V (bf16)
            QT = acts.tile([128, QH, TT, 128], bf16)    # [d, h, tt, t]
            KT = acts.tile([128, TT, 128], bf16)        # [d, kt, t]
            Vt = acts.tile([128, TT, 128], bf16)        # [t, kt, d]

            def attn_head(qc, h):
                pv = pvp.tile([128, 512], f32, tag="pv")
                sm = smp.tile([128, 512], f32, tag="sm")
                nkt = 4 * (qc + 1)
                for kt in range(nkt):
                    # diagonal band: only q-subtiles j >= m are visible
                    m = max(kt - 4 * qc, 0)
                    lo = m * 128
                    ss = sps.tile([128, 512], f32, tag="ss")
                    nc.tensor.matmul(ss[:, lo:512], KT[:, kt, :],
                                     QT[:, h, 4 * qc + m:4 * qc + 4, :],
                                     start=True, stop=True)
                    pT = probs.tile([128, 4, 128], bf16, tag="pT")
                    pTf = pT.rearrange("p a b -> p (a b)")
                    nc.scalar.activation(pTf[:, lo:512], ss[:, lo:512],
                                         ACT.Exp, scale=SCALE)
                    if kt >= 4 * qc:
                        nc.gpsimd.tensor_tensor(pT[:, m, :], pT[:, m, :],
                                                triu_sb[:], OP.mult)
                    nc.tensor.matmul(pv[:, lo:512], Vt[:, kt, :],
                                     pTf[:, lo:512],
                                     start=(kt == 0), stop=(kt == nkt - 1))
                    nc.tensor.matmul(sm[:, lo:512], ones_sb[:],
                                     pTf[:, lo:512],
                                     start=(kt == 0), stop=(kt == nkt - 1))
                recip = attn.tile([128, 512], f32, tag="recip")
                nc.vector.reciprocal_approx_fast(recip[:], sm[:])
                at = attn.tile([128, 512], bf16, tag="at")
                nc.vector.tensor_tensor(at[:], pv[:], recip[:], OP.mult)
                if qc == QC - 1:
                    nc.sync.dma_start(
                        ag3_in[h // 2][(h % 2) * 128:(h % 2 + 1) * 128, :],
                        at[:])
                else:
                    nc.sync.dma_start(ag_in[qc][h * 128:(h + 1) * 128, :],
                                      at[:])

            def attn_gather(qc):
                # AllGather this query block's attention output across cores
                nc.gpsimd.collective_compute(
                    "AllGather", mybir.AluOpType.bypass,
                    replica_groups=[list(range(N_CORES))],
                    ins=[ag_in[qc].opt()], outs=[ag_out[qc].opt()])

            def attn_gather3(half):
                nc.gpsimd.collective_compute(
                    "AllGather", mybir.AluOpType.bypass,
                    replica_groups=[list(range(N_CORES))],
                    ins=[ag3_in[half].opt()], outs=[ag3_out[half].opt()])

            with tc.tile_pool(name="htp", bufs=4) as htp, \
                 tc.tile_pool(name="qkps", bufs=1, space="PSUM") as qkps, \
                 tc.tile_pool(name="p1t", bufs=2) as p1t:

                # startup DMA order: first hidden tile + first weight chunk
                # first so the qkv matmuls can start ASAP
                ht0 = htp.tile([128, KO, 128], bf16, tag="ht")
                nc.sync.dma_start(ht0[:, 0:KO // 2, :], hT.ap()[0][:, 0:KO // 2, :])
                nc.sync.dma_start(ht0[:, KO // 2:, :], hT.ap()[0][:, KO // 2:, :])
                wqkv_r = wqkv.ap().rearrange("(ko p) n -> p ko n", p=128)
                wqkv_sb = htp.tile([128, KO, 768], bf16, tag="wqkv", bufs=1)
                # fine-grained chunks so matmuls unblock per-chunk as the
                # weight stream lands
                for c in range(16):
                    nc.sync.dma_start(wqkv_sb[:, 2 * c:2 * (c + 1), :],
                                      wqkv_r[:, 2 * c:2 * (c + 1), :])

                cos_sb = const.tile([128, TT, 64], f32)
                nc.sync.dma_start(cos_sb[:], cosd.ap())
                sin_sb = const.tile([128, TT, 64], f32)
                nc.sync.dma_start(sin_sb[:], sind.ap())
                lnw_sb = const.tile([128, LNH, 128], f32)
                nc.sync.dma_start(lnw_sb[:], lnw.ap())
                triu_sb = const.tile([128, 128], bf16)
                nc.sync.dma_start(triu_sb[:], triu.ap())
                ident_sb = const.tile([128, 128], bf16)
                nc.sync.dma_start(ident_sb[:], ident.ap())
                ones_sb = const.tile([128, 128], bf16)
                nc.sync.dma_start(ones_sb[:], onesd.ap())

                # eps bias for the LN sqrt (sum-of-squares space: HD*eps)
                eps_sb = const.tile([128, 1], f32)
                nc.vector.memset(eps_sb[:], HD * EPS)

                # prime the exp activation table while DMAs stream
                prime = const.tile([128, 2], f32)
                nc.vector.memset(prime[:, 0:1], 1.0)
                nc.scalar.activation(prime[:, 1:2], prime[:, 0:1], ACT.Exp,
                                     scale=1.0)

                for tt in range(TT):
                    if tt == 0:
                        ht_t = ht0
                    else:
                        ht_t = htp.tile([128, KO, 128], bf16, tag="ht")
                        nc.sync.dma_start(ht_t[:, 0:KO // 2, :],
                                          hT.ap()[tt][:, 0:KO // 2, :])
                        nc.sync.dma_start(ht_t[:, KO // 2:, :],
                                          hT.ap()[tt][:, KO // 2:, :])
                    ps = qkps.tile([128, 768], f32, tag="qk")
                    for ko in range(KO):
                        nc.tensor.matmul(ps[:, 0:512], ht_t[:, ko, :],
                                         wqkv_sb[:, ko, 0:512],
                                         start=(ko == 0), stop=(ko == KO - 1))
                        nc.tensor.matmul(ps[:, 512:768], ht_t[:, ko, :],
                                         wqkv_sb[:, ko, 512:768],
                                         start=(ko == 0), stop=(ko == KO - 1))

                    # V: cast psum -> persistent bf16 tile on the scalar engine
                    # (gpsimd cannot read PSUM; scalar only runs exps otherwise)
                    nc.scalar.copy(Vt[:, tt, :], ps[:, 640:768])

                    # layernorm over the 5 q/k heads, DVE only, from PSUM.
                    # xc = mean - x (negated; sign restored by negated lnw)
                    x5 = ps[:, 0:640].rearrange("p (h d) -> p h d", d=128)
                    sum5 = p1t.tile([128, LNH], f32, tag="sum5")
                    nc.vector.tensor_reduce(sum5[:], x5, AX.X, OP.add)
                    xc = p1t.tile([128, LNH, 128], f32, tag="xc")
                    nc.vector.scalar_tensor_tensor(
                        xc[:], sum5[:, :, None].to_broadcast((128, LNH, 128)),
                        1.0 / HD, x5, OP.mult, OP.subtract)
                    sq = p1t.tile([128, LNH, 128], f32, tag="sq")
                    nc.vector.tensor_tensor(sq[:], xc[:], xc[:], OP.mult)
                    vs = p1t.tile([128, LNH], f32, tag="vs")
                    nc.vector.tensor_reduce(vs[:], sq[:], AX.X, OP.add)
                    # rstd' = (sum_sq + HD*eps)^-0.5 via DVE-only fast inverse
                    # sqrt (bit-trick seed + 2 Newton steps, ~5e-6 rel err).
                    # Any scalar-engine Sqrt/Ln here would force two 1.28us
                    # activation-table reloads per section right before the
                    # attention exps. The missing sqrt(HD) factor is folded
                    # into the host-side lnw.
                    tv = p1t.tile([128, LNH], f32, tag="tv")
                    nc.vector.tensor_scalar(tv[:], vs[:], HD * EPS, None,
                                            OP.add)
                    sd = p1t.tile([128, LNH], dt.int32, tag="sd")
                    nc.vector.tensor_scalar(sd[:], tv[:].bitcast(dt.int32), 1,
                                            None, OP.arith_shift_right)
                    nc.vector.tensor_scalar(sd[:], sd[:], -1, 0x5f3759df,
                                            OP.mult, OP.add)
                    y0 = sd[:].bitcast(f32)
                    aa = p1t.tile([128, LNH], f32, tag="aa")
                    rstd = p1t.tile([128, LNH], f32, tag="rstd")
                    nc.vector.tensor_tensor(aa[:], y0, y0, OP.mult)
                    nc.vector.tensor_tensor(aa[:], aa[:], tv[:], OP.mult)
                    nc.vector.tensor_scalar(aa[:], aa[:], -0.5, 1.5,
                                            OP.mult, OP.add)
                    nc.vector.tensor_tensor(rstd[:], y0, aa[:], OP.mult)
                    nc.vector.tensor_tensor(aa[:], rstd[:], rstd[:], OP.mult)
                    nc.vector.tensor_tensor(aa[:], aa[:], tv[:], OP.mult)
                    nc.vector.tensor_scalar(aa[:], aa[:], -0.5, 1.5,
                                            OP.mult, OP.add)
                    nc.vector.tensor_tensor(rstd[:], rstd[:], aa[:], OP.mult)
                    nc.vector.tensor_tensor(
                        xc[:], xc[:], rstd[:, :, None].to_broadcast((128, LNH, 128)),
                        OP.mult)
                    nc.vector.tensor_tensor(xc[:], xc[:], lnw_sb[:], OP.mult)

                    # interleaved RoPE: out[2i] = x1*cos - x2*sin; out[2i+1] = x2*cos + x1*sin
                    x1 = xc[:, :, 0:128:2]
                    x2 = xc[:, :, 1:128:2]
                    cos_b = cos_sb[:, tt:tt + 1, :].to_broadcast((128, LNH, 64))
                    sin_b = sin_sb[:, tt:tt + 1, :].to_broadcast((128, LNH, 64))
                    m1 = p1t.tile([128, LNH, 64], f32, tag="m1")
                    m2 = p1t.tile([128, LNH, 64], f32, tag="m2")
                    qkf = p1t.tile([128, LNH, 128], bf16, tag="qkf")
                    nc.vector.tensor_tensor(m1[:], x1, cos_b, OP.mult)
                    nc.vector.tensor_tensor(m2[:], x2, sin_b, OP.mult)
                    nc.vector.tensor_tensor(qkf[:, :, 0:128:2], m1[:], m2[:], OP.subtract)
                    nc.vector.tensor_tensor(m1[:], x2, cos_b, OP.mult)
                    nc.vector.tensor_tensor(m2[:], x1, sin_b, OP.mult)
                    nc.vector.tensor_tensor(qkf[:, :, 1:128:2], m1[:], m2[:], OP.add)

                    # attention heads trickled per section (block qc's QT/KT
                    # complete after section 4qc+3). Block 2 runs two heads
                    # per section so its AllGather fires two sections early:
                    # the tail is serialized on the collective engine
                    # (AG2 -> AG3a -> AG3b gates the last o_proj block).
                    for qc_h, h_h in HEAD_SCHED.get(tt, []):
                        attn_head(qc_h, h_h)

                    # transpose each head tile [t,d] -> [d,t]
                    for h5 in range(LNH):
                        pst = sps.tile([128, 128], bf16, tag="ss")
                        nc.tensor.transpose(pst[:], qkf[:, h5, :], ident_sb[:])
                        if h5 < QH:
                            nc.vector.tensor_copy(QT[:, h5, tt, :], pst[:])
                        else:
                            nc.vector.tensor_copy(KT[:, tt, :], pst[:])

                    if tt in (7, 11, 13):
                        attn_gather({7: 0, 11: 1, 13: 2}[tt])

            # w_o loaded late so its DMA doesn't delay the P1 weight loads
            wo_r = wo.ap().rearrange("(ko p) n -> p ko n", p=128)
            wo_sb = const.tile([128, KO, 512], bf16)
            for c in range(4):
                nc.sync.dma_start(wo_sb[:, 8 * c:8 * (c + 1), :],
                                  wo_r[:, 8 * c:8 * (c + 1), :])

            # last block's heads + split gather (fire half after 2 heads)
            attn_head(QC - 1, 0)
            attn_head(QC - 1, 1)
            attn_gather3(0)
            attn_head(QC - 1, 2)
            attn_head(QC - 1, 3)
            attn_gather3(1)

            # ---- P5: o_proj ----
            with tc.tile_pool(name="agp", bufs=3) as agp, \
                 tc.tile_pool(name="osb", bufs=3) as osb, \
                 tc.tile_pool(name="ops", bufs=2, space="PSUM") as ops:

                def oproj_block(tq):
                    # (chunk_source, sbuf_ko_slot, wo_ko_index) per 128-row
                    # contraction chunk; block 3 is gathered as two 2-head
                    # halves so its rows are permuted: half a chunk j holds
                    # core j//2's head j%2 (+2 for half b) = wo row block
                    # 4*(j//2) + (j%2) [+2].
                    rt = agp.tile([128, KO, 512], bf16, tag="rt")
                    if tq == QC - 1:
                        for half in range(2):
                            agr = ag3_out[half].rearrange(
                                "(ko p) n -> p ko n", p=128)
                            for c in range(4):
                                nc.sync.dma_start(
                                    rt[:, 16 * half + 4 * c:
                                       16 * half + 4 * (c + 1), :],
                                    agr[:, 4 * c:4 * (c + 1), :])
                        ko_map = [4 * (j // 2) + (j % 2) for j in range(16)] \
                            + [4 * (j // 2) + 2 + (j % 2) for j in range(16)]
                    else:
                        agr = ag_out[tq].rearrange("(ko p) n -> p ko n", p=128)
                        for c in range(8):
                            nc.sync.dma_start(rt[:, 4 * c:4 * (c + 1), :],
                                              agr[:, 4 * c:4 * (c + 1), :])
                        ko_map = list(range(KO))
                    for hc in range(4):
                        po = ops.tile([128, 512], f32, tag="po")
                        for ko in range(KO):
                            nc.tensor.matmul(po[:],
                                             wo_sb[:, ko_map[ko],
                                                   hc * 128:(hc + 1) * 128],
                                             rt[:, ko, :],
                                             start=(ko == 0), stop=(ko == KO - 1))
                        ot = osb.tile([128, 512], f32, tag="ot")
                        nc.scalar.copy(ot[:], po[:])
                        nc.sync.dma_start(
                            outT.ap()[hc * 128:(hc + 1) * 128,
                                      tq * 512:(tq + 1) * 512],
                            ot[:])

                for tq in range(QC):
                    oproj_block(tq)

    nc.compile()
    return nc


def _prep_inputs(positions, hidden_states, w_qkv, w_o, q_norm_w, k_norm_w):
    hidden_states = np.asarray(hidden_states, dtype=np.float32)
    w_qkv = np.asarray(w_qkv, dtype=np.float32)
    w_o = np.asarray(w_o, dtype=np.float32)
    q_norm_w = np.asarray(q_norm_w, dtype=np.float32)
    k_norm_w = np.asarray(k_norm_w, dtype=np.float32)
    pos = np.asarray(positions).astype(np.float32)

    # hiddenT tiled for 8KB-contiguous per-partition DMA: [tt, p(H%128), ko, tl]
    hTd = np.ascontiguousarray(
        hidden_states.reshape(TT, 128, KO, 128).transpose(0, 3, 2, 1)
    ).astype(BF16)

    inv_freq = THETA ** (-np.arange(64, dtype=np.float32) / 64.0)
    freqs = pos[:, None] * inv_freq[None, :]
    cos = np.cos(freqs).astype(np.float32).reshape(TT, 128, 64).transpose(1, 0, 2)
    sin = np.sin(freqs).astype(np.float32).reshape(TT, 128, 64).transpose(1, 0, 2)
    cos = np.ascontiguousarray(cos)
    sin = np.ascontiguousarray(sin)

    triu = np.triu(np.ones((128, 128), dtype=np.float32)).astype(BF16)
    identm = np.eye(128, dtype=np.float32).astype(BF16)
    onesm = np.ones((128, 128), dtype=np.float32).astype(BF16)

    in_maps = []
    for c in range(N_CORES):
        qcols = w_qkv[:, 4 * c * HD:(4 * c + 4) * HD]
        kcols = w_qkv[:, NH * HD + c * HD: NH * HD + (c + 1) * HD]
        vcols = w_qkv[:, (NH + NKV) * HD + c * HD: (NH + NKV) * HD + (c + 1) * HD]
        wqkv_sh = np.concatenate([qcols, kcols, vcols], axis=1).astype(BF16)
        wo_sh = np.ascontiguousarray(w_o[:, 512 * c:512 * (c + 1)]).astype(BF16)
        ln5 = np.concatenate([q_norm_w[4 * c:4 * c + 4], k_norm_w[c:c + 1]], axis=0)
        # negated and scaled by sqrt(HD): compensates the negated mean-centered
        # x and the rstd computed without the 1/HD normalization on-device
        ln5 = ln5 * (-np.sqrt(float(HD)))
        lnw_rep = np.ascontiguousarray(
            np.broadcast_to(ln5[None, :, :], (128, LNH, 128))).astype(np.float32)
        in_maps.append({
            "hT": hTd,
            "wqkv": wqkv_sh,
            "wo": wo_sh,
            "cosd": cos,
            "sind": sin,
            "lnw": lnw_rep,
            "triu": triu,
            "ident": identm,
            "onesd": onesm,
        })
    return in_maps


def kernel(positions, hidden_states, w_qkv, w_o, q_norm_w, k_norm_w):
    from concourse.bass_utils import run_bass_kernel_spmd

    if "nc" not in _CACHE:
        _CACHE["nc"] = _build()
    nc = _CACHE["nc"]

    in_maps = _prep_inputs(positions, hidden_states, w_qkv, w_o,
                           q_norm_w, k_norm_w)
    res = run_bass_kernel_spmd(nc, in_maps, core_ids=list(range(N_CORES)))
    out = np.empty((T, H), dtype=np.float32)
    for c in range(N_CORES):
        out[:, 512 * c:512 * (c + 1)] = res.results[c]["outT"].T
    return out


# revision 26
# speedup vs baseline: 1.0268x; 1.0268x over previous
"""Trainium2 Bass kernel for CohereAttention (T=2048, H=4096, NH=32, NKV=8, HD=128).

Sharding: tensor-parallel across heads on 8 cores (SGLang-style).
  - core c owns q-heads [4c, 4c+4) and kv-head c (GQA rep=4 maps exactly).
  - w_qkv column-sharded per core -> [4096, 768] (4q|1k|1v head blocks).
  - attention output (bf16, d-major [512, 2048]) AllGather'd across cores.
  - w_o column-sharded -> each core computes a [2048, 512] column shard of the
    output (stored transposed [512, 2048]); host concatenates.

Device pipeline per core (engine-decoupled, one attention head trickled per
token-tile section so the PE never starves on the DVE layernorm chain):
  section tt: qkv MMs (PSUM, single buffer) -> V cast (gpsimd) ->
      LN+RoPE chain (DVE only; mean/var scale and sqrt folded into
      scalar_tensor_tensor / tensor_scalar-pow, sign+sqrt(HD) folded into the
      host-prepped lnw) -> one attention head of the newest ready q-block
      (scores MM -> exp on Scalar -> causal mask on gpsimd -> PV/sums MMs,
      epilogue: reciprocal_approx_fast on DVE + normalize mult on gpsimd) ->
      PE transposes of q/k tiles -> (AllGather when a block's 4 heads done).
  tail: w_o load, last block's 4 heads, AllGather, o_proj per gathered block.
"""

import numpy as np
import ml_dtypes

T = 2048
H = 4096
NH = 32
NKV = 8
HD = 128
N_CORES = 8
QH = NH // N_CORES          # q heads per core = 4
LNH = QH + 1                # layernormed heads per core (4 q + 1 k)
EPS = 1e-5
THETA = 10000.0
SCALE = HD ** -0.5
TT = T // 128               # 16 token tiles
KO = H // 128               # 32 contraction chunks
QC = T // 512               # 4 query chunks of 512
BF16 = ml_dtypes.bfloat16

# section -> attention heads (qc, h) to emit there; block 2 front-loaded
HEAD_SCHED = {
    4: [(0, 0)], 5: [(0, 1)], 6: [(0, 2)], 7: [(0, 3)],
    8: [(1, 0)], 9: [(1, 1)], 10: [(1, 2)], 11: [(1, 3)],
    12: [(2, 0), (2, 1)], 13: [(2, 2), (2, 3)],
}

_CACHE = {}


def _build():
    import concourse.bass as bass
    import concourse.mybir as mybir
    import concourse.tile as tile
    from concourse import bacc
    from contextlib import ExitStack

    dt = mybir.dt
    f32 = dt.float32
    bf16 = dt.bfloat16
    f8 = dt.float8e4
    AX = mybir.AxisListType
    OP = mybir.AluOpType
    ACT = mybir.ActivationFunctionType

    nc = bacc.Bacc("TRN2", target_bir_lowering=False, debug=False,
                   num_devices=N_CORES)

    # ---- I/O ----
    hT = nc.dram_tensor("hT", [TT, 128, KO, 128], bf16, kind="ExternalInput")
    wqkv = nc.dram_tensor("wqkv", [H, 768], bf16, kind="ExternalInput")
    wo = nc.dram_tensor("wo", [H, 512], bf16, kind="ExternalInput")
    cosd = nc.dram_tensor("cosd", [128, TT, 64], f32, kind="ExternalInput")
    sind = nc.dram_tensor("sind", [128, TT, 64], f32, kind="ExternalInput")
    lnw = nc.dram_tensor("lnw", [128, LNH, 128], f32, kind="ExternalInput")
    triu = nc.dram_tensor("triu", [128, 128], bf16, kind="ExternalInput")
    ident = nc.dram_tensor("ident", [128, 128], bf16, kind="ExternalInput")
    onesd = nc.dram_tensor("onesd", [128, 128], bf16, kind="ExternalInput")
    outT = nc.dram_tensor("outT", [512, T], f32, kind="ExternalOutput")

    with tile.TileContext(nc) as tc, ExitStack() as ctx:
        const = ctx.enter_context(tc.tile_pool(name="const", bufs=1))
        dram = ctx.enter_context(tc.tile_pool(name="dram", bufs=1, space="DRAM"))

        ag_in = [dram.tile([QH * 128, 512], bf16, name=f"agi{i}")
                 for i in range(QC)]
        ag_out = [dram.tile([NH * 128, 512], bf16, addr_space="Shared",
                            name=f"ago{i}")
                  for i in range(QC)]
        # the last block's gather is split in two 2-head halves so o_proj's
        # final block can start ~half an AllGather earlier
        ag3_in = [dram.tile([2 * 128, 512], bf16, name=f"agi3{i}")
                  for i in range(2)]
        ag3_out = [dram.tile([N_CORES * 2 * 128, 512], bf16,
                             addr_space="Shared", name=f"ago3{i}")
                   for i in range(2)]

        with tc.tile_pool(name="sps", bufs=2, space="PSUM") as sps, \
             tc.tile_pool(name="pvp", bufs=2, space="PSUM") as pvp, \
             tc.tile_pool(name="smp", bufs=2, space="PSUM") as smp, \
             tc.tile_pool(name="probs", bufs=3) as probs, \
             tc.tile_pool(name="attn", bufs=3) as attn, \
             tc.tile_pool(name="acts", bufs=1) as acts:

            # persistent activations: d-major Q/K, t-major V (bf16)
            QT = acts.tile([128, QH, TT, 128], bf16)    # [d, h, tt, t]
            KT = acts.tile([128, TT, 128], bf16)        # [d, kt, t]
            Vt = acts.tile([128, TT, 128], bf16)        # [t, kt, d]

            def attn_head(qc, h):
                # scores/exp/PV per 128-key tile (bf16); the softmax-denominator
                # "sums" matmul runs on fp8 probs pairs with DoubleRow packing
                # (K=256 per pass) to halve its PE streaming cost. Probs are
                # cast bf16->fp8 on the DVE; denominator error from e4m3 probs
                # measured at ~1.1e-2 rel (budget 2e-2).
                pv = pvp.tile([128, 512], f32, tag="pv")
                sm = smp.tile([128, 512], f32, tag="sm")
                nkt = 4 * (qc + 1)
                for u in range(nkt // 2):
                    p8 = probs.tile([128, 2, 512], f8, tag="p8")
                    lo_pair = 0
                    for par in range(2):
                        kt = 2 * u + par
                        # diagonal band: only q-subtiles j >= m are visible
                        m = max(kt - 4 * qc, 0)
                        lo = m * 128
                        if par == 0:
                            lo_pair = lo
                        elif lo > lo_pair:
                            # zero the masked strip so the fp8 pair matmul can
                            # stream both tiles over the union q-range
                            nc.vector.memset(p8[:, 1, lo_pair:lo], 0.0)
                        ss = sps.tile([128, 512], f32, tag="ss")
                        nc.tensor.matmul(ss[:, lo:512], KT[:, kt, :],
                                         QT[:, h, 4 * qc + m:4 * qc + 4, :],
                                         start=True, stop=True)
                        pT = probs.tile([128, 4, 128], bf16, tag="pT")
                        pTf = pT.rearrange("p a b -> p (a b)")
                        nc.scalar.activation(pTf[:, lo:512], ss[:, lo:512],
                                             ACT.Exp, scale=SCALE)
                        if kt >= 4 * qc:
                            nc.gpsimd.tensor_tensor(pT[:, m, :], pT[:, m, :],
                                                    triu_sb[:], OP.mult)
                        nc.vector.tensor_copy(p8[:, par, lo:512],
                                              pTf[:, lo:512])
                        nc.tensor.matmul(pv[:, lo:512], Vt[:, kt, :],
                                         pTf[:, lo:512],
                                         start=(kt == 0), stop=(kt == nkt - 1))
                    nc.tensor.matmul(sm[:, lo_pair:512], ones8_sb[:],
                                     p8[:, :, lo_pair:512],
                                     start=(u == 0), stop=(u == nkt // 2 - 1),
                                     perf_mode=mybir.MatmulPerfMode.DoubleRow)
                recip = attn.tile([128, 512], f32, tag="recip")
                nc.vector.reciprocal_approx_fast(recip[:], sm[:])
                at = attn.tile([128, 512], bf16, tag="at")
                nc.vector.tensor_tensor(at[:], pv[:], recip[:], OP.mult)
                if qc == QC - 1:
                    nc.sync.dma_start(
                        ag3_in[h // 2][(h % 2) * 128:(h % 2 + 1) * 128, :],
                        at[:])
                else:
                    nc.sync.dma_start(ag_in[qc][h * 128:(h + 1) * 128, :],
                                      at[:])

            def attn_gather(qc):
                # AllGather this query block's attention output across cores
                nc.gpsimd.collective_compute(
                    "AllGather", mybir.AluOpType.bypass,
                    replica_groups=[list(range(N_CORES))],
                    ins=[ag_in[qc].opt()], outs=[ag_out[qc].opt()])

            def attn_gather3(half):
                nc.gpsimd.collective_compute(
                    "AllGather", mybir.AluOpType.bypass,
                    replica_groups=[list(range(N_CORES))],
                    ins=[ag3_in[half].opt()], outs=[ag3_out[half].opt()])

            with tc.tile_pool(name="htp", bufs=4) as htp, \
                 tc.tile_pool(name="qkps", bufs=1, space="PSUM") as qkps, \
                 tc.tile_pool(name="p1t", bufs=2) as p1t:

                # startup DMA order: first hidden tile + first weight chunk
                # first so the qkv matmuls can start ASAP
                ht0 = htp.tile([128, KO, 128], bf16, tag="ht")
                nc.sync.dma_start(ht0[:, 0:KO // 2, :], hT.ap()[0][:, 0:KO // 2, :])
                nc.sync.dma_start(ht0[:, KO // 2:, :], hT.ap()[0][:, KO // 2:, :])
                wqkv_r = wqkv.ap().rearrange("(ko p) n -> p ko n", p=128)
                wqkv_sb = htp.tile([128, KO, 768], bf16, tag="wqkv", bufs=1)
                # fine-grained chunks so matmuls unblock per-chunk as the
                # weight stream lands
                for c in range(16):
                    nc.sync.dma_start(wqkv_sb[:, 2 * c:2 * (c + 1), :],
                                      wqkv_r[:, 2 * c:2 * (c + 1), :])

                cos_sb = const.tile([128, TT, 64], f32)
                nc.sync.dma_start(cos_sb[:], cosd.ap())
                sin_sb = const.tile([128, TT, 64], f32)
                nc.sync.dma_start(sin_sb[:], sind.ap())
                lnw_sb = const.tile([128, LNH, 128], f32)
                nc.sync.dma_start(lnw_sb[:], lnw.ap())
                triu_sb = const.tile([128, 128], bf16)
                nc.sync.dma_start(triu_sb[:], triu.ap())
                ident_sb = const.tile([128, 128], bf16)
                nc.sync.dma_start(ident_sb[:], ident.ap())
                ones_sb = const.tile([128, 128], bf16)
                nc.sync.dma_start(ones_sb[:], onesd.ap())
                ones8_sb = const.tile([128, 2, 128], f8)
                nc.vector.memset(ones8_sb[:], 1.0)

                # eps bias for the LN sqrt (sum-of-squares space: HD*eps)
                eps_sb = const.tile([128, 1], f32)
                nc.vector.memset(eps_sb[:], HD * EPS)

                # prime the exp activation table while DMAs stream
                prime = const.tile([128, 2], f32)
                nc.vector.memset(prime[:, 0:1], 1.0)
                nc.scalar.activation(prime[:, 1:2], prime[:, 0:1], ACT.Exp,
                                     scale=1.0)

                for tt in range(TT):
                    if tt == 0:
                        ht_t = ht0
                    else:
                        ht_t = htp.tile([128, KO, 128], bf16, tag="ht")
                        nc.sync.dma_start(ht_t[:, 0:KO // 2, :],
                                          hT.ap()[tt][:, 0:KO // 2, :])
                        nc.sync.dma_start(ht_t[:, KO // 2:, :],
                                          hT.ap()[tt][:, KO // 2:, :])
                    ps = qkps.tile([128, 768], f32, tag="qk")
                    for ko in range(KO):
                        nc.tensor.matmul(ps[:, 0:512], ht_t[:, ko, :],
                                         wqkv_sb[:, ko, 0:512],
                                         start=(ko == 0), stop=(ko == KO - 1))
                        nc.tensor.matmul(ps[:, 512:768], ht_t[:, ko, :],
                                         wqkv_sb[:, ko, 512:768],
                                         start=(ko == 0), stop=(ko == KO - 1))

                    # V: cast psum -> persistent bf16 tile on the scalar engine
                    # (gpsimd cannot read PSUM; scalar only runs exps otherwise)
                    nc.scalar.copy(Vt[:, tt, :], ps[:, 640:768])

                    # layernorm over the 5 q/k heads, DVE only, from PSUM.
                    # xc = mean - x (negated; sign restored by negated lnw)
                    x5 = ps[:, 0:640].rearrange("p (h d) -> p h d", d=128)
                    sum5 = p1t.tile([128, LNH], f32, tag="sum5")
                    nc.vector.tensor_reduce(sum5[:], x5, AX.X, OP.add)
                    xc = p1t.tile([128, LNH, 128], f32, tag="xc")
                    nc.vector.scalar_tensor_tensor(
                        xc[:], sum5[:, :, None].to_broadcast((128, LNH, 128)),
                        1.0 / HD, x5, OP.mult, OP.subtract)
                    sq = p1t.tile([128, LNH, 128], f32, tag="sq")
                    nc.vector.tensor_tensor(sq[:], xc[:], xc[:], OP.mult)
                    vs = p1t.tile([128, LNH], f32, tag="vs")
                    nc.vector.tensor_reduce(vs[:], sq[:], AX.X, OP.add)
                    # rstd' = (sum_sq + HD*eps)^-0.5 via DVE-only fast inverse
                    # sqrt (bit-trick seed + 2 Newton steps, ~5e-6 rel err).
                    # Any scalar-engine Sqrt/Ln here would force two 1.28us
                    # activation-table reloads per section right before the
                    # attention exps. The missing sqrt(HD) factor is folded
                    # into the host-side lnw.
                    tv = p1t.tile([128, LNH], f32, tag="tv")
                    nc.vector.tensor_scalar(tv[:], vs[:], HD * EPS, None,
                                            OP.add)
                    sd = p1t.tile([128, LNH], dt.int32, tag="sd")
                    nc.vector.tensor_scalar(sd[:], tv[:].bitcast(dt.int32), 1,
                                            None, OP.arith_shift_right)
                    nc.vector.tensor_scalar(sd[:], sd[:], -1, 0x5f3759df,
                                            OP.mult, OP.add)
                    y0 = sd[:].bitcast(f32)
                    aa = p1t.tile([128, LNH], f32, tag="aa")
                    rstd = p1t.tile([128, LNH], f32, tag="rstd")
                    nc.vector.tensor_tensor(aa[:], y0, y0, OP.mult)
                    nc.vector.tensor_tensor(aa[:], aa[:], tv[:], OP.mult)
                    nc.vector.tensor_scalar(aa[:], aa[:], -0.5, 1.5,
                                            OP.mult, OP.add)
                    nc.vector.tensor_tensor(rstd[:], y0, aa[:], OP.mult)
                    nc.vector.tensor_tensor(aa[:], rstd[:], rstd[:], OP.mult)
                    nc.vector.tensor_tensor(aa[:], aa[:], tv[:], OP.mult)
                    nc.vector.tensor_scalar(aa[:], aa[:], -0.5, 1.5,
                                            OP.mult, OP.add)
                    nc.vector.tensor_tensor(rstd[:], rstd[:], aa[:], OP.mult)
                    nc.vector.tensor_tensor(
                        xc[:], xc[:], rstd[:, :, None].to_broadcast((128, LNH, 128)),
                        OP.mult)
                    nc.vector.tensor_tensor(xc[:], xc[:], lnw_sb[:], OP.mult)

                    # interleaved RoPE: out[2i] = x1*cos - x2*sin; out[2i+1] = x2*cos + x1*sin
                    x1 = xc[:, :, 0:128:2]
                    x2 = xc[:, :, 1:128:2]
                    cos_b = cos_sb[:, tt:tt + 1, :].to_broadcast((128, LNH, 64))
                    sin_b = sin_sb[:, tt:tt + 1, :].to_broadcast((128, LNH, 64))
                    m1 = p1t.tile([128, LNH, 64], f32, tag="m1")
                    m2 = p1t.tile([128, LNH, 64], f32, tag="m2")
                    qkf = p1t.tile([128, LNH, 128], bf16, tag="qkf")
                    nc.vector.tensor_tensor(m1[:], x1, cos_b, OP.mult)
                    nc.vector.tensor_tensor(m2[:], x2, sin_b, OP.mult)
                    nc.vector.tensor_tensor(qkf[:, :, 0:128:2], m1[:], m2[:], OP.subtract)
                    nc.vector.tensor_tensor(m1[:], x2, cos_b, OP.mult)
                    nc.vector.tensor_tensor(m2[:], x1, sin_b, OP.mult)
                    nc.vector.tensor_tensor(qkf[:, :, 1:128:2], m1[:], m2[:], OP.add)

                    # attention heads trickled per section (block qc's QT/KT
                    # complete after section 4qc+3). Block 2 runs two heads
                    # per section so its AllGather fires two sections early:
                    # the tail is serialized on the collective engine
                    # (AG2 -> AG3a -> AG3b gates the last o_proj block).
                    for qc_h, h_h in HEAD_SCHED.get(tt, []):
                        attn_head(qc_h, h_h)

                    # transpose each head tile [t,d] -> [d,t]
                    for h5 in range(LNH):
                        pst = sps.tile([128, 128], bf16, tag="ss")
                        nc.tensor.transpose(pst[:], qkf[:, h5, :], ident_sb[:])
                        if h5 < QH:
                            nc.vector.tensor_copy(QT[:, h5, tt, :], pst[:])
                        else:
                            nc.vector.tensor_copy(KT[:, tt, :], pst[:])

                    if tt in (7, 11, 13):
                        attn_gather({7: 0, 11: 1, 13: 2}[tt])

            # w_o loaded late so its DMA doesn't delay the P1 weight loads
            wo_r = wo.ap().rearrange("(ko p) n -> p ko n", p=128)
            wo_sb = const.tile([128, KO, 512], bf16)
            for c in range(4):
                nc.sync.dma_start(wo_sb[:, 8 * c:8 * (c + 1), :],
                                  wo_r[:, 8 * c:8 * (c + 1), :])

            # last block's heads + split gather (fire half after 2 heads)
            attn_head(QC - 1, 0)
            attn_head(QC - 1, 1)
            attn_gather3(0)
            attn_head(QC - 1, 2)
            attn_head(QC - 1, 3)
            attn_gather3(1)

            # ---- P5: o_proj ----
            with tc.tile_pool(name="agp", bufs=3) as agp, \
                 tc.tile_pool(name="osb", bufs=3) as osb, \
                 tc.tile_pool(name="ops", bufs=2, space="PSUM") as ops:

                def oproj_block(tq):
                    # (chunk_source, sbuf_ko_slot, wo_ko_index) per 128-row
                    # contraction chunk; block 3 is gathered as two 2-head
                    # halves so its rows are permuted: half a chunk j holds
                    # core j//2's head j%2 (+2 for half b) = wo row block
                    # 4*(j//2) + (j%2) [+2].
                    rt = agp.tile([128, KO, 512], bf16, tag="rt")
                    if tq == QC - 1:
                        for half in range(2):
                            agr = ag3_out[half].rearrange(
                                "(ko p) n -> p ko n", p=128)
                            for c in range(4):
                                nc.sync.dma_start(
                                    rt[:, 16 * half + 4 * c:
                                       16 * half + 4 * (c + 1), :],
                                    agr[:, 4 * c:4 * (c + 1), :])
                        ko_map = [4 * (j // 2) + (j % 2) for j in range(16)] \
                            + [4 * (j // 2) + 2 + (j % 2) for j in range(16)]
                    else:
                        agr = ag_out[tq].rearrange("(ko p) n -> p ko n", p=128)
                        for c in range(8):
                            nc.sync.dma_start(rt[:, 4 * c:4 * (c + 1), :],
                                              agr[:, 4 * c:4 * (c + 1), :])
                        ko_map = list(range(KO))
                    for hc in range(4):
                        po = ops.tile([128, 512], f32, tag="po")
                        for ko in range(KO):
                            nc.tensor.matmul(po[:],
                                             wo_sb[:, ko_map[ko],
                                                   hc * 128:(hc + 1) * 128],
                                             rt[:, ko, :],
                                             start=(ko == 0), stop=(ko == KO - 1))
                        ot = osb.tile([128, 512], f32, tag="ot")
                        nc.scalar.copy(ot[:], po[:])
                        nc.sync.dma_start(
                            outT.ap()[hc * 128:(hc + 1) * 128,
                                      tq * 512:(tq + 1) * 512],
                            ot[:])

                for tq in range(QC):
                    oproj_block(tq)

    nc.compile()
    return nc


def _prep_inputs(positions, hidden_states, w_qkv, w_o, q_norm_w, k_norm_w):
    hidden_states = np.asarray(hidden_states, dtype=np.float32)
    w_qkv = np.asarray(w_qkv, dtype=np.float32)
    w_o = np.asarray(w_o, dtype=np.float32)
    q_norm_w = np.asarray(q_norm_w, dtype=np.float32)
    k_norm_w = np.asarray(k_norm_w, dtype=np.float32)
    pos = np.asarray(positions).astype(np.float32)

    # hiddenT tiled for 8KB-contiguous per-partition DMA: [tt, p(H%128), ko, tl]
    hTd = np.ascontiguousarray(
        hidden_states.reshape(TT, 128, KO, 128).transpose(0, 3, 2, 1)
    ).astype(BF16)

    inv_freq = THETA ** (-np.arange(64, dtype=np.float32) / 64.0)
    freqs = pos[:, None] * inv_freq[None, :]
    cos = np.cos(freqs).astype(np.float32).reshape(TT, 128, 64).transpose(1, 0, 2)
    sin = np.sin(freqs).astype(np.float32).reshape(TT, 128, 64).transpose(1, 0, 2)
    cos = np.ascontiguousarray(cos)
    sin = np.ascontiguousarray(sin)

    triu = np.triu(np.ones((128, 128), dtype=np.float32)).astype(BF16)
    identm = np.eye(128, dtype=np.float32).astype(BF16)
    onesm = np.ones((128, 128), dtype=np.float32).astype(BF16)

    in_maps = []
    for c in range(N_CORES):
        qcols = w_qkv[:, 4 * c * HD:(4 * c + 4) * HD]
        kcols = w_qkv[:, NH * HD + c * HD: NH * HD + (c + 1) * HD]
        vcols = w_qkv[:, (NH + NKV) * HD + c * HD: (NH + NKV) * HD + (c + 1) * HD]
        wqkv_sh = np.concatenate([qcols, kcols, vcols], axis=1).astype(BF16)
        wo_sh = np.ascontiguousarray(w_o[:, 512 * c:512 * (c + 1)]).astype(BF16)
        ln5 = np.concatenate([q_norm_w[4 * c:4 * c + 4], k_norm_w[c:c + 1]], axis=0)
        # negated and scaled by sqrt(HD): compensates the negated mean-centered
        # x and the rstd computed without the 1/HD normalization on-device
        ln5 = ln5 * (-np.sqrt(float(HD)))
        lnw_rep = np.ascontiguousarray(
            np.broadcast_to(ln5[None, :, :], (128, LNH, 128))).astype(np.float32)
        in_maps.append({
            "hT": hTd,
            "wqkv": wqkv_sh,
            "wo": wo_sh,
            "cosd": cos,
            "sind": sin,
            "lnw": lnw_rep,
            "triu": triu,
            "ident": identm,
            "onesd": onesm,
        })
    return in_maps


def kernel(positions, hidden_states, w_qkv, w_o, q_norm_w, k_norm_w):
    from concourse.bass_utils import run_bass_kernel_spmd

    if "nc" not in _CACHE:
        _CACHE["nc"] = _build()
    nc = _CACHE["nc"]

    in_maps = _prep_inputs(positions, hidden_states, w_qkv, w_o,
                           q_norm_w, k_norm_w)
    res = run_bass_kernel_spmd(nc, in_maps, core_ids=list(range(N_CORES)))
    out = np.empty((T, H), dtype=np.float32)
    for c in range(N_CORES):
        out[:, 512 * c:512 * (c + 1)] = res.results[c]["outT"].T
    return out
